# revision 1
# baseline (speedup 1.0000x reference)
"""Causal self-attention (B=2, T=2048, C=1024, H=16) on 8 TRN2 NeuronCores.

Sharding: tensor-parallel over heads (2 heads/core) for QKV projection and
attention; AllToAll converts the head-sharded attention output into a
sequence-sharded layout; each core then computes its 512-row slice of the
output projection. Host only slices/casts inputs and concatenates outputs.

Device math in bf16 with fp32 PSUM accumulation:
  - x is pre-transposed on host to xT [C, B*T] (bf16) so every matmul
    contraction has channels on the partition axis.
  - Scores are built transposed (S^T [keys, queries]) so softmax
    normalization sums arrive for free from a ones-augmented P^T @ [V|1]
    matmul, and no on-device transposes are needed anywhere.
  - exp on ScalarE (fp32-accurate LUT); no max-subtraction needed since
    scores are O(+-8).
"""
import os
import math
import threading

import numpy as np
import ml_dtypes

import concourse.bass as bass
import concourse.tile as tile
from concourse import mybir, bacc, bass_utils

B, T, C, H = 2, 2048, 1024, 16
D = C // H                 # 64
NCORES = 8
HPC = H // NCORES          # heads per core = 2
HC = HPC * D               # head-channels per core = 128
BT = B * T                 # 4096
TQ = 512                   # query chunk
TKT = 128                  # key tile
ROWS = BT // NCORES        # output rows per core = 512
SM_SCALE = 1.0 / math.sqrt(D)

F32 = mybir.dt.float32
BF16 = mybir.dt.bfloat16
BF16_NP = ml_dtypes.bfloat16


def _build_program():
    nc = bacc.Bacc("TRN2", target_bir_lowering=False, debug=False,
                   num_devices=NCORES)
    xt = nc.dram_tensor("xt", [C, BT], BF16, kind="ExternalInput").ap()
    wqkv = nc.dram_tensor("wqkv", [C, 3 * HC], BF16, kind="ExternalInput").ap()
    wproj = nc.dram_tensor("wproj", [C, C], BF16, kind="ExternalInput").ap()
    bq = nc.dram_tensor("bq", [HC, 1], F32, kind="ExternalInput").ap()
    bk = nc.dram_tensor("bk", [HC, 1], F32, kind="ExternalInput").ap()
    bv = nc.dram_tensor("bv", [1, HC], BF16, kind="ExternalInput").ap()
    bproj = nc.dram_tensor("bproj", [1, C], BF16, kind="ExternalInput").ap()
    masks = nc.dram_tensor("masks", [TQ // TKT, TKT, TQ], BF16,
                           kind="ExternalInput").ap()
    outp = nc.dram_tensor("out", [ROWS, C], F32, kind="ExternalOutput").ap()

    KT = C // 128          # 8 contraction tiles over channels
    NCH = BT // TQ         # 8 T-chunks over B*T
    SPC = TQ // D          # 8 strips of 64 rows per chunk (one per core)

    with tile.TileContext(nc) as tc:
        with (
            tc.tile_pool(name="consts", bufs=1) as consts,
            tc.tile_pool(name="xpool", bufs=2) as xpool,
            tc.tile_pool(name="ppool", bufs=6) as ppool,
            tc.tile_pool(name="npool", bufs=2) as npool,
            tc.tile_pool(name="opool", bufs=2) as opool,
            tc.tile_pool(name="ps_o", bufs=2, space="PSUM") as ps_o,
            tc.tile_pool(name="dram", bufs=1, space="DRAM") as dram,
        ):
            # ---- stage 0: weights & constants ----
            wqkv_sb = []
            for kt in range(KT):
                w1 = consts.tile([128, 3 * HC], BF16, name=f"wqkv_sb{kt}")
                nc.sync.dma_start(out=w1, in_=wqkv[128 * kt:128 * (kt + 1), :])
                wqkv_sb.append(w1)
            # big weights not needed until ~100us in: keep them off the SP
            # HWDGE queue so the stage-1 stream starts immediately
            wproj_sb = []
            for kt in range(KT):
                w2 = consts.tile([128, C], BF16, name=f"wproj_sb{kt}")
                nc.gpsimd.dma_start(out=w2, in_=wproj[128 * kt:128 * (kt + 1), :])
                wproj_sb.append(w2)
            ones_sb = consts.tile([1, 128], BF16, name="ones_sb")
            nc.vector.memset(ones_sb, 1.0)
            bq_sb = consts.tile([HC, 1], F32, name="bq_sb")
            nc.sync.dma_start(out=bq_sb, in_=bq)
            bk_sb = consts.tile([HC, 1], F32, name="bk_sb")
            nc.sync.dma_start(out=bk_sb, in_=bk)
            bv_sb = consts.tile([1, HC], BF16, name="bv_sb")
            nc.sync.dma_start(out=bv_sb, in_=bv)
            bproj_sb = consts.tile([1, C], BF16, name="bproj_sb")
            nc.sync.dma_start(out=bproj_sb, in_=bproj)
            masks_sb = consts.tile([TKT, TQ // TKT, TQ], BF16, name="masks_sb")
            nc.gpsimd.dma_start(out=masks_sb, in_=masks.rearrange("r p q -> p r q"))

            qT_b = [consts.tile([HC, T], BF16, name=f"qT_sb{b}")
                    for b in range(B)]
            kT_b = [consts.tile([HC, T], BF16, name=f"kT_sb{b}")
                    for b in range(B)]
            v_sb = [consts.tile([128, 2 * (D + 1)], BF16, name=f"v_sb{tt}")
                    for tt in range(BT // 128)]

            # per-chunk exchange buffers: block s of chunk c = queries
            # [64s, 64s+64) of that chunk, owned by core s
            a2a_in = [dram.tile([NCORES, HC, D], BF16, name=f"a2a_in{c}")
                      for c in range(NCH)]
            a2a_out = [dram.tile([NCORES, HC, D], BF16, name=f"a2a_out{c}")
                       for c in range(NCH)]

            def stage4_pair(cA, cB):
                """Output projection for two 64-row strips, column-packed."""
                ylhs = {}
                for ci, c in enumerate((cA, cB)):
                    yy = opool.tile([128, SPC, D], BF16, tag=f"ylhs{ci}",
                                    name=f"ylhs{ci}")
                    nc.sync.dma_start(
                        out=yy, in_=a2a_out[c].rearrange("k p q -> p k q"))
                    ylhs[c] = yy
                for n in range(C // TQ):
                    po = ps_o.tile([128, TQ], F32, tag="po", name="po")
                    for ci, c in enumerate((cA, cB)):
                        pslice = po[D * ci:D * (ci + 1), :]
                        for kt in range(KT):
                            nc.tensor.matmul(
                                pslice,
                                lhsT=ylhs[c][:, kt, :],
                                rhs=wproj_sb[kt][:, TQ * n:TQ * (n + 1)],
                                start=(kt == 0), stop=False)
                        nc.tensor.matmul(
                            pslice, lhsT=ones_sb[:, 0:D],
                            rhs=bproj_sb[:, TQ * n:TQ * (n + 1)],
                            start=False, stop=True)
                    osb = opool.tile([128, TQ], F32, tag="osb", name="osb")
                    nc.vector.tensor_copy(out=osb, in_=po)
                    for ci, c in enumerate((cA, cB)):
                        nc.sync.dma_start(
                            out=outp[D * c:D * (c + 1), TQ * n:TQ * (n + 1)],
                            in_=osb[D * ci:D * (ci + 1), :])

            done_chunks = []
            for b in range(B):
                # ---- stage 1: QKV projection for this batch ----
                with (
                    tc.tile_pool(name=f"ps_qk{b}", bufs=3, space="PSUM") as ps_qk,
                    tc.tile_pool(name=f"ps_v{b}", bufs=2, space="PSUM") as ps_v,
                ):
                    for cl in range(NCH // B):
                        c = (NCH // B) * b + cl
                        xt_t = []
                        for kt in range(KT):
                            xx = xpool.tile([128, TQ], BF16, tag=f"xt{kt}")
                            nc.sync.dma_start(
                                out=xx,
                                in_=xt[128 * kt:128 * (kt + 1),
                                       TQ * c:TQ * (c + 1)])
                            xt_t.append(xx)
                        for which, off, bias, scale in (
                            ("q", 0, bq_sb, 1.0),
                            ("k", HC, bk_sb, SM_SCALE),
                        ):
                            ps = ps_qk.tile([HC, TQ], F32, tag="qk")
                            for kt in range(KT):
                                nc.tensor.matmul(
                                    ps,
                                    lhsT=wqkv_sb[kt][:, off:off + HC],
                                    rhs=xt_t[kt],
                                    start=(kt == 0), stop=(kt == KT - 1))
                            dst = qT_b[b] if which == "q" else kT_b[b]
                            nc.scalar.activation(
                                out=dst[:, TQ * cl:TQ * (cl + 1)], in_=ps,
                                func=mybir.ActivationFunctionType.Identity,
                                bias=bias, scale=scale)
                        # V (natural layout, ones-augmented)
                        for s in range(TQ // 128):
                            tt = 4 * c + s
                            ps = ps_v.tile([128, HC], F32, tag="v")
                            for kt in range(KT):
                                nc.tensor.matmul(
                                    ps,
                                    lhsT=xt_t[kt][:, 128 * s:128 * (s + 1)],
                                    rhs=wqkv_sb[kt][:, 2 * HC:3 * HC],
                                    start=(kt == 0), stop=False)
                            nc.tensor.matmul(ps, lhsT=ones_sb, rhs=bv_sb,
                                             start=False, stop=True)
                            vt = v_sb[tt]
                            nc.vector.tensor_copy(out=vt[:, 0:D], in_=ps[:, 0:D])
                            nc.vector.tensor_copy(out=vt[:, D + 1:2 * D + 1],
                                                  in_=ps[:, D:2 * D])
                            nc.vector.memset(vt[:, D:D + 1], 1.0)
                            nc.vector.memset(vt[:, 2 * D + 1:2 * D + 2], 1.0)

                # ---- stage 2: attention for this batch, largest chunks
                # first; each chunk's exchange + output projection follows
                # immediately and hides under later chunks' attention ----
                with (
                    tc.tile_pool(name=f"ps_s{b}", bufs=4, space="PSUM") as ps_s,
                    tc.tile_pool(name=f"ps_y{b}", bufs=1, space="PSUM") as ps_y,
                ):
                    for jl in reversed(range(T // TQ)):
                        cidx = (T // TQ) * b + jl
                        q0 = TQ * jl
                        nkt = (TQ // TKT) * (jl + 1)
                        y_ps = [ps_y.tile([D + 1, TQ], F32, tag=f"y{h}",
                                          name=f"y_ps{h}")
                                for h in range(HPC)]
                        pts = []
                        for kt in range(nkt):
                            k0 = TKT * kt
                            r = kt - (TQ // TKT) * jl
                            pt_pair = []
                            for h in range(HPC):
                                hp = D * h
                                ss = ps_s.tile([TKT, TQ], F32, tag="s",
                                               name=f"ss{h}")
                                nc.tensor.matmul(
                                    ss,
                                    lhsT=kT_b[b][hp:hp + D, k0:k0 + TKT],
                                    rhs=qT_b[b][hp:hp + D, q0:q0 + TQ],
                                    start=True, stop=True)
                                pt = ppool.tile([TKT, TQ], BF16, tag=f"pt{h}",
                                                name=f"pt{h}")
                                nc.scalar.activation(
                                    out=pt, in_=ss,
                                    func=mybir.ActivationFunctionType.Exp)
                                if r >= 0:
                                    nc.vector.tensor_mul(pt, pt,
                                                         masks_sb[:, r, :])
                                pt_pair.append(pt)
                            pts.append(pt_pair)
                        for kt in range(nkt):
                            vt = v_sb[(T // 128) * b + kt]
                            for h in range(HPC):
                                nc.tensor.matmul(
                                    y_ps[h],
                                    lhsT=vt[:, (D + 1) * h:(D + 1) * (h + 1)],
                                    rhs=pts[kt][h],
                                    start=(kt == 0), stop=(kt == nkt - 1))
                        for h in range(HPC):
                            recip = npool.tile([1, TQ], F32, tag="recip")
                            nc.vector.reciprocal(recip, y_ps[h][D:D + 1, :])
                            recip_b = npool.tile([D, TQ], F32, tag="recipb")
                            nc.gpsimd.partition_broadcast(recip_b, recip)
                            yt = npool.tile([D, TQ], BF16, tag="yt")
                            nc.vector.tensor_mul(yt, y_ps[h][0:D, :], recip_b)
                            nc.sync.dma_start(
                                out=a2a_in[cidx][:, D * h:D * (h + 1), :]
                                    .rearrange("s p q -> p s q"),
                                in_=yt.rearrange("p (s q) -> p s q", s=SPC))
                        nc.gpsimd.collective_compute(
                            "AllToAll", mybir.AluOpType.bypass,
                            replica_groups=[list(range(NCORES))],
                            ins=[a2a_in[cidx].opt()],
                            outs=[a2a_out[cidx].opt()])
                        done_chunks.append(cidx)
                        if len(done_chunks) % 2 == 0:
                            stage4_pair(done_chunks[-2], done_chunks[-1])

    nc.compile()
    return nc


_lock = threading.Lock()
_cached_nc = None
last_results = None  # BassKernelResults of the most recent kernel() call


def _get_program():
    global _cached_nc
    with _lock:
        if _cached_nc is None:
            _cached_nc = _build_program()
    return _cached_nc


def _host_inputs(x, W_qkv, b_qkv, W_proj, b_proj):
    bf = lambda a: np.ascontiguousarray(a).astype(BF16_NP)
    x = np.asarray(x, dtype=np.float32)
    W_qkv = np.asarray(W_qkv, dtype=np.float32)
    b_qkv = np.asarray(b_qkv, dtype=np.float32)
    W_proj = np.asarray(W_proj, dtype=np.float32)
    b_proj = np.asarray(b_proj, dtype=np.float32)

    xt = bf(x.reshape(BT, C).T)                     # [C, BT]
    wproj = bf(W_proj)                              # [C, C]
    bproj = bf(b_proj.reshape(1, C))
    r = np.arange(TQ // TKT)[:, None, None]
    k = np.arange(TKT)[None, :, None]
    q = np.arange(TQ)[None, None, :]
    masks = ((k + TKT * r) <= q).astype(BF16_NP)    # [4, 128, 512]

    in_maps = []
    for i in range(NCORES):
        sel = slice(HC * i, HC * (i + 1))
        wq = W_qkv[:, sel]
        wk = W_qkv[:, C + HC * i:C + HC * (i + 1)]
        wv = W_qkv[:, 2 * C + HC * i:2 * C + HC * (i + 1)]
        in_maps.append({
            "xt": xt,
            "wqkv": bf(np.concatenate([wq, wk, wv], axis=1)),
            "wproj": wproj,
            "bq": np.ascontiguousarray(
                b_qkv[sel].reshape(HC, 1)).astype(np.float32),
            "bk": np.ascontiguousarray(
                (b_qkv[C + HC * i:C + HC * (i + 1)] * SM_SCALE)
                .reshape(HC, 1)).astype(np.float32),
            "bv": b_qkv[2 * C + HC * i:2 * C + HC * (i + 1)]
                .reshape(1, HC).astype(BF16_NP),
            "bproj": bproj,
            "masks": masks,
        })
    return in_maps


def kernel(x, W_qkv, b_qkv, W_proj, b_proj):
    global last_results
    nc = _get_program()
    in_maps = _host_inputs(x, W_qkv, b_qkv, W_proj, b_proj)
    trace = bool(int(os.environ.get("KERNEL_TRACE", "0")))
    res = bass_utils.run_bass_kernel_spmd(
        nc, in_maps, core_ids=list(range(NCORES)), trace=trace)
    last_results = res
    # core s's output rows are strip s (64 rows) of every 512-row chunk
    arr = np.stack([res.results[s]["out"].reshape(BT // TQ, D, C)
                    for s in range(NCORES)], axis=1)   # [chunk, core, 64, C]
    return np.ascontiguousarray(arr.reshape(B, T, C))



# revision 3
# speedup vs baseline: 1.4373x; 1.4373x over previous
"""Causal self-attention (B=2, T=2048, C=1024, H=16) on 8 TRN2 NeuronCores.

Sharding: tensor-parallel over heads (2 heads/core) for QKV projection and
attention; AllToAll converts the head-sharded attention output into a
sequence-sharded layout; each core then computes its 512-row slice of the
output projection. Host only slices/casts inputs and concatenates outputs.

Device math in bf16 with fp32 PSUM accumulation:
  - x is pre-transposed on host to xT [C, B*T] (bf16) so every matmul
    contraction has channels on the partition axis.
  - Scores are built transposed (S^T [keys, queries]); both heads' score
    matmuls are row-tiled (K=64 halves of the PE array) and run
    concurrently; both land in one 2-bank PSUM tile so a single exp
    covers both heads.
  - PV matmuls are col-tiled (M=64 halves) and run concurrently.
  - Softmax sums: P tiles are accumulated on DVE; one col-tiled
    broadcast-matmul pair per chunk produces [128,512] broadcast sums,
    so normalization is two full-width DVE ops (reciprocal + mul).
  - Causal slicing: diagonal key-tiles only compute/exp the valid
    query-column range; a single [128,2,128] tril slab handles masking.
  - Output projection packs two 64-row query strips into M=128 matmuls;
    biases are added by fused DVE adds (no bias matmuls).
"""
import os
import math
import threading

import numpy as np
import ml_dtypes

import concourse.bass as bass
import concourse.tile as tile
from concourse import mybir, bacc, bass_utils

B, T, C, H = 2, 2048, 1024, 16
D = C // H                 # 64
NCORES = 8
HPC = H // NCORES          # heads per core = 2
HC = HPC * D               # head-channels per core = 128
BT = B * T                 # 4096
TQ = 512                   # query chunk
TKT = 128                  # key tile
ROWS = BT // NCORES        # output rows per core = 512
SM_SCALE = 1.0 / math.sqrt(D)
KT = C // 128              # 8 contraction tiles over channels
NCH = BT // TQ             # 8 T-chunks over B*T
SPC = TQ // D              # 8 strips of 64 rows per chunk (one per core)

F32 = mybir.dt.float32
BF16 = mybir.dt.bfloat16
BF16_NP = ml_dtypes.bfloat16


def _build_program():
    nc = bacc.Bacc("TRN2", target_bir_lowering=False, debug=False,
                   num_devices=NCORES)
    xt = nc.dram_tensor("xt", [C, BT], BF16, kind="ExternalInput").ap()
    wqkv = nc.dram_tensor("wqkv", [C, 3 * HC], BF16, kind="ExternalInput").ap()
    wproj = nc.dram_tensor("wproj", [C, C], BF16, kind="ExternalInput").ap()
    bq = nc.dram_tensor("bq", [HC, 1], F32, kind="ExternalInput").ap()
    bk = nc.dram_tensor("bk", [HC, 1], F32, kind="ExternalInput").ap()
    bvb = nc.dram_tensor("bvb", [128, HC], BF16, kind="ExternalInput").ap()
    bpb = nc.dram_tensor("bpb", [128, C], BF16, kind="ExternalInput").ap()
    mask2 = nc.dram_tensor("mask2", [TKT, 2, TKT], BF16,
                           kind="ExternalInput").ap()
    outp = nc.dram_tensor("out", [ROWS, C], F32, kind="ExternalOutput").ap()

    with tile.TileContext(nc) as tc:
        with (
            tc.tile_pool(name="consts", bufs=1) as consts,
            tc.tile_pool(name="xpool", bufs=2) as xpool,
            tc.tile_pool(name="ppool", bufs=4) as ppool,
            tc.tile_pool(name="apool", bufs=3) as apool,
            tc.tile_pool(name="npool", bufs=2) as npool,
            tc.tile_pool(name="opool", bufs=2) as opool,
            tc.tile_pool(name="ps_ss", bufs=2, space="PSUM") as ps_ss,
            tc.tile_pool(name="ps_y", bufs=2, space="PSUM") as ps_y,
            tc.tile_pool(name="ps_sum", bufs=1, space="PSUM") as ps_sum,
            tc.tile_pool(name="ps_o", bufs=1, space="PSUM") as ps_o,
            tc.tile_pool(name="dram", bufs=1, space="DRAM") as dram,
        ):
            # ---- stage 0: weights & constants ----
            bq_sb = consts.tile([HC, 1], F32, name="bq_sb")
            nc.sync.dma_start(out=bq_sb, in_=bq)
            bk_sb = consts.tile([HC, 1], F32, name="bk_sb")
            nc.sync.dma_start(out=bk_sb, in_=bk)
            bvb_sb = consts.tile([128, HC], BF16, name="bvb_sb")
            nc.sync.dma_start(out=bvb_sb, in_=bvb)
            # wqkv split per-section so the q slices (needed first) arrive
            # first on the queue
            wqkv_sb = []
            for kt in range(KT):
                w1 = consts.tile([128, 3 * HC], BF16, name=f"wqkv_sb{kt}")
                wqkv_sb.append(w1)
            for sec in range(3):
                cs = slice(HC * sec, HC * (sec + 1))
                for kt in range(KT):
                    nc.sync.dma_start(
                        out=wqkv_sb[kt][:, cs],
                        in_=wqkv[128 * kt:128 * (kt + 1), cs])
            # big weights not needed until late: keep them off the sync queue
            wproj_sb = []
            for kt in range(KT):
                w2 = consts.tile([128, C], BF16, name=f"wproj_sb{kt}")
                nc.gpsimd.dma_start(out=w2, in_=wproj[128 * kt:128 * (kt + 1), :])
                wproj_sb.append(w2)
            bpb_sb = consts.tile([128, C], BF16, name="bpb_sb")
            nc.gpsimd.dma_start(out=bpb_sb, in_=bpb)
            mask_sb = consts.tile([TKT, 2, TKT], BF16, name="mask_sb")
            nc.gpsimd.dma_start(out=mask_sb, in_=mask2)
            ones64 = consts.tile([128, D], BF16, name="ones64")
            nc.vector.memset(ones64, 1.0)

            qT_b = [consts.tile([HC, T], BF16, name=f"qT_sb{b}")
                    for b in range(B)]
            kT_b = [consts.tile([HC, T], BF16, name=f"kT_sb{b}")
                    for b in range(B)]
            # v tiles: [128 keys, 128] = [v_h0 (64) | v_h1 (64)]
            v_sb = [consts.tile([128, HC], BF16, name=f"v_sb{tt}")
                    for tt in range(BT // 128)]

            # per-chunk exchange buffers: block s of chunk c = queries
            # [64s, 64s+64) of that chunk, owned by core s
            a2a_in = [dram.tile([NCORES, HC, D], BF16, name=f"a2a_in{c}")
                      for c in range(NCH)]
            a2a_out = [dram.tile([NCORES, HC, D], BF16, name=f"a2a_out{c}")
                       for c in range(NCH)]

            # ---- stage 1: QKV projection (both batches up front) ----
            for b in range(B):
                for cp in range(2):                      # chunk pairs
                    xx = []
                    for kt in range(KT):
                        x1 = xpool.tile([128, 2 * TQ], BF16, tag=f"xt{kt}")
                        nc.sync.dma_start(
                            out=x1,
                            in_=xt[128 * kt:128 * (kt + 1),
                                   1024 * (2 * b + cp):1024 * (2 * b + cp) + 1024])
                        xx.append(x1)
                    for half in range(2):
                        cl = 2 * cp + half               # chunk in batch
                        c = 4 * b + cl                   # global chunk
                        col = slice(TQ * half, TQ * (half + 1))
                        ps = ps_ss.tile([128, 2 * TQ], F32, tag="ss")
                        for kt in range(KT):
                            nc.tensor.matmul(
                                ps[:, 0:TQ],
                                lhsT=wqkv_sb[kt][:, 0:HC],
                                rhs=xx[kt][:, col],
                                start=(kt == 0), stop=(kt == KT - 1))
                        for kt in range(KT):
                            nc.tensor.matmul(
                                ps[:, TQ:2 * TQ],
                                lhsT=wqkv_sb[kt][:, HC:2 * HC],
                                rhs=xx[kt][:, col],
                                start=(kt == 0), stop=(kt == KT - 1))
                        nc.scalar.activation(
                            out=qT_b[b][:, TQ * cl:TQ * (cl + 1)],
                            in_=ps[:, 0:TQ],
                            func=mybir.ActivationFunctionType.Identity,
                            bias=bq_sb, scale=1.0)
                        nc.scalar.activation(
                            out=kT_b[b][:, TQ * cl:TQ * (cl + 1)],
                            in_=ps[:, TQ:2 * TQ],
                            func=mybir.ActivationFunctionType.Identity,
                            bias=bk_sb, scale=SM_SCALE)
                        for s in range(TQ // 128):
                            tt = 4 * c + s
                            pv = ps_sum.tile([128, HC], F32, tag="sum")
                            for kt in range(KT):
                                nc.tensor.matmul(
                                    pv,
                                    lhsT=xx[kt][:, TQ * half + 128 * s:
                                                TQ * half + 128 * (s + 1)],
                                    rhs=wqkv_sb[kt][:, 2 * HC:3 * HC],
                                    start=(kt == 0), stop=(kt == KT - 1))
                            nc.vector.tensor_add(v_sb[tt], pv, bvb_sb)

            # ---- stage 2: attention; two chunks in flight, biggest first;
            # each pair's exchange + output projection follows immediately
            # and hides under the next pair's attention ----
            def emit_S(b, jl, slot):
                """Issue both heads' score matmuls + exp (+ mask) for key
                tile `slot` of chunk jl; returns (pt tile, c0)."""
                i = slot
                q0 = TQ * jl
                k0 = TKT * i
                r = i - (TQ // TKT) * jl
                c0 = TKT * r if r >= 0 else 0
                ss = ps_ss.tile([128, 2 * TQ], F32, tag="ss")
                nc.tensor.matmul(
                    ss[:, c0:TQ],
                    lhsT=kT_b[b][0:D, k0:k0 + TKT],
                    rhs=qT_b[b][0:D, q0 + c0:q0 + TQ],
                    start=True, stop=True)
                nc.tensor.matmul(
                    ss[:, TQ + c0:2 * TQ],
                    lhsT=kT_b[b][D:2 * D, k0:k0 + TKT],
                    rhs=qT_b[b][D:2 * D, q0 + c0:q0 + TQ],
                    start=True, stop=True)
                pt = ppool.tile([128, 2, TQ], BF16, tag="pt")
                ssv = ss.rearrange("p (h q) -> p h q", h=2)
                nc.scalar.activation(
                    out=pt[:, :, c0:TQ], in_=ssv[:, :, c0:TQ],
                    func=mybir.ActivationFunctionType.Exp)
                if r >= 0:
                    nc.vector.tensor_mul(pt[:, :, c0:c0 + TKT],
                                         pt[:, :, c0:c0 + TKT], mask_sb)
                return pt, c0

            def emit_PV(b, jl, i, nkt, y_ps, acc, pt, c0):
                vt = v_sb[(T // 128) * b + i]
                ptf = pt.rearrange("p h q -> p (h q)")
                nc.tensor.matmul(
                    y_ps[0:D, c0:TQ],
                    lhsT=vt[:, 0:D],
                    rhs=ptf[:, c0:TQ],
                    start=(i == 0), stop=(i == nkt - 1),
                    skip_group_check=True)
                nc.tensor.matmul(
                    y_ps[D:2 * D, c0:TQ],
                    lhsT=vt[:, D:2 * D],
                    rhs=ptf[:, TQ + c0:2 * TQ],
                    start=(i == 0), stop=(i == nkt - 1),
                    skip_group_check=True)
                if i == 0:
                    nc.vector.tensor_copy(out=acc, in_=ptf)
                else:
                    accv = acc.rearrange("p (h q) -> p h q", h=2)
                    nc.vector.tensor_add(accv[:, :, c0:TQ],
                                         accv[:, :, c0:TQ],
                                         pt[:, :, c0:TQ])

            def finalize(b, jl, cidx, y_ps, acc):
                sums = ps_sum.tile([128, TQ], F32, tag="sum")
                nc.tensor.matmul(sums[0:D, :], lhsT=ones64,
                                 rhs=acc[:, 0:TQ], start=True, stop=True)
                nc.tensor.matmul(sums[D:2 * D, :], lhsT=ones64,
                                 rhs=acc[:, TQ:2 * TQ], start=True, stop=True)
                recip = npool.tile([128, TQ], F32, tag="recip")
                nc.vector.reciprocal(recip, sums)
                yt = npool.tile([128, TQ], BF16, tag="yt")
                nc.vector.tensor_mul(yt, y_ps, recip)
                nc.gpsimd.dma_start(
                    out=a2a_in[cidx].rearrange("s p q -> p s q"),
                    in_=yt.rearrange("p (s q) -> p s q", s=SPC))
                nc.gpsimd.collective_compute(
                    "AllToAll", mybir.AluOpType.bypass,
                    replica_groups=[list(range(NCORES))],
                    ins=[a2a_in[cidx].opt()],
                    outs=[a2a_out[cidx].opt()])

            def stage4_pair(cA, cB):
                """Output projection for two 64-row strips, M=128-packed."""
                yy = opool.tile([128, KT, 2 * D], BF16, tag="ylhs")
                nc.gpsimd.dma_start(
                    out=yy[:, :, 0:D],
                    in_=a2a_out[cA].rearrange("k p q -> p k q"))
                nc.gpsimd.dma_start(
                    out=yy[:, :, D:2 * D],
                    in_=a2a_out[cB].rearrange("k p q -> p k q"))
                for n in range(C // TQ):
                    po = ps_o.tile([128, TQ], F32, tag="po")
                    for kt in range(KT):
                        nc.tensor.matmul(
                            po,
                            lhsT=yy[:, kt, :],
                            rhs=wproj_sb[kt][:, TQ * n:TQ * (n + 1)],
                            start=(kt == 0), stop=(kt == KT - 1))
                    osb = opool.tile([128, TQ], F32, tag="osb")
                    nc.vector.tensor_add(osb, po,
                                         bpb_sb[:, TQ * n:TQ * (n + 1)])
                    for ci, cc in enumerate((cA, cB)):
                        nc.sync.dma_start(
                            out=outp[D * cc:D * (cc + 1), TQ * n:TQ * (n + 1)],
                            in_=osb[D * ci:D * (ci + 1), :])

            for b in range(B):
                for jA, jB in ((3, 2), (1, 0)):
                    cA, cB = 4 * b + jA, 4 * b + jB
                    nktA, nktB = 4 * (jA + 1), 4 * (jB + 1)
                    yA = ps_y.tile([128, TQ], F32, tag="y", name="yA")
                    yB = ps_y.tile([128, TQ], F32, tag="y", name="yB")
                    accA = apool.tile([128, 2 * TQ], BF16, tag="acc",
                                      name="accA")
                    accB = apool.tile([128, 2 * TQ], BF16, tag="acc",
                                      name="accB")
                    prevA = prevB = None
                    for i in range(nktA):
                        curA = emit_S(b, jA, i)
                        if prevA is not None:
                            emit_PV(b, jA, i - 1, nktA, yA, accA, *prevA)
                        prevA = curA
                        if i < nktB:
                            curB = emit_S(b, jB, i)
                            if prevB is not None:
                                emit_PV(b, jB, i - 1, nktB, yB, accB, *prevB)
                            prevB = curB
                    emit_PV(b, jA, nktA - 1, nktA, yA, accA, *prevA)
                    emit_PV(b, jB, nktB - 1, nktB, yB, accB, *prevB)
                    finalize(b, jA, cA, yA, accA)
                    finalize(b, jB, cB, yB, accB)
                    stage4_pair(cA, cB)

    nc.compile()
    return nc


_lock = threading.Lock()
_cached_nc = None
last_results = None  # BassKernelResults of the most recent kernel() call


def _get_program():
    global _cached_nc
    with _lock:
        if _cached_nc is None:
            _cached_nc = _build_program()
    return _cached_nc


def _host_inputs(x, W_qkv, b_qkv, W_proj, b_proj):
    bf = lambda a: np.ascontiguousarray(a).astype(BF16_NP)
    x = np.asarray(x, dtype=np.float32)
    W_qkv = np.asarray(W_qkv, dtype=np.float32)
    b_qkv = np.asarray(b_qkv, dtype=np.float32)
    W_proj = np.asarray(W_proj, dtype=np.float32)
    b_proj = np.asarray(b_proj, dtype=np.float32)

    xt = bf(x.reshape(BT, C).T)                     # [C, BT]
    wproj = bf(W_proj)                              # [C, C]
    bpb = bf(np.broadcast_to(b_proj.reshape(1, C), (128, C)))
    k = np.arange(TKT)[:, None, None]
    q = np.arange(TKT)[None, None, :]
    mask2 = np.broadcast_to(k <= q, (TKT, 2, TKT)).astype(BF16_NP)

    in_maps = []
    for i in range(NCORES):
        sel = slice(HC * i, HC * (i + 1))
        wq = W_qkv[:, sel]
        wk = W_qkv[:, C + HC * i:C + HC * (i + 1)]
        wv = W_qkv[:, 2 * C + HC * i:2 * C + HC * (i + 1)]
        bv = b_qkv[2 * C + HC * i:2 * C + HC * (i + 1)]
        in_maps.append({
            "xt": xt,
            "wqkv": bf(np.concatenate([wq, wk, wv], axis=1)),
            "wproj": wproj,
            "bq": np.ascontiguousarray(
                b_qkv[sel].reshape(HC, 1)).astype(np.float32),
            "bk": np.ascontiguousarray(
                (b_qkv[C + HC * i:C + HC * (i + 1)] * SM_SCALE)
                .reshape(HC, 1)).astype(np.float32),
            "bvb": bf(np.broadcast_to(bv.reshape(1, HC), (128, HC))),
            "bpb": bpb,
            "mask2": np.ascontiguousarray(mask2),
        })
    return in_maps


def kernel(x, W_qkv, b_qkv, W_proj, b_proj):
    global last_results
    nc = _get_program()
    in_maps = _host_inputs(x, W_qkv, b_qkv, W_proj, b_proj)
    trace = bool(int(os.environ.get("KERNEL_TRACE", "0")))
    res = bass_utils.run_bass_kernel_spmd(
        nc, in_maps, core_ids=list(range(NCORES)), trace=trace)
    last_results = res
    # core s's output rows are strip s (64 rows) of every 512-row chunk
    arr = np.stack([res.results[s]["out"].reshape(BT // TQ, D, C)
                    for s in range(NCORES)], axis=1)   # [chunk, core, 64, C]
    return np.ascontiguousarray(arr.reshape(B, T, C))


# revision 7
# speedup vs baseline: 1.5459x; 1.0756x over previous
"""Causal self-attention (B=2, T=2048, C=1024, H=16) on 8 TRN2 NeuronCores.

Sharding: tensor-parallel over heads (2 heads/core) for QKV projection and
attention; AllToAll converts the head-sharded attention output into a
sequence-sharded layout; each core then computes its 512-row slice of the
output projection. Host only slices/casts inputs and concatenates outputs.

Device math in bf16 with fp32 PSUM accumulation:
  - x is pre-transposed on host to xT [C, B*T] (bf16) so every matmul
    contraction has channels on the partition axis.
  - Scores are built transposed (S^T [keys, queries]); both heads' score
    matmuls are row-tiled (K=64 halves of the PE array) and run
    concurrently; both land in one 2-bank PSUM tile so a single exp
    covers both heads.
  - PV matmuls are col-tiled (M=64 halves) and run concurrently.
  - Softmax sums: P tiles are accumulated on DVE; one col-tiled
    broadcast-matmul pair per chunk produces [128,512] broadcast sums,
    so normalization is two full-width DVE ops (reciprocal + mul).
  - Causal slicing: diagonal key-tiles only compute/exp the valid
    query-column range; a single [128,2,128] tril slab handles masking.
  - Output projection packs two 64-row query strips into M=128 matmuls;
    biases are added by fused DVE adds (no bias matmuls).
"""
import os
import math
import threading

import numpy as np
import ml_dtypes

import concourse.bass as bass
import concourse.tile as tile
from concourse import mybir, bacc, bass_utils

B, T, C, H = 2, 2048, 1024, 16
D = C // H                 # 64
NCORES = 8
HPC = H // NCORES          # heads per core = 2
HC = HPC * D               # head-channels per core = 128
BT = B * T                 # 4096
TQ = 512                   # query chunk
TKT = 128                  # key tile
ROWS = BT // NCORES        # output rows per core = 512
SM_SCALE = 1.0 / math.sqrt(D)
KT = C // 128              # 8 contraction tiles over channels
NCH = BT // TQ             # 8 T-chunks over B*T
SPC = TQ // D              # 8 strips of 64 rows per chunk (one per core)

F32 = mybir.dt.float32
BF16 = mybir.dt.bfloat16
BF16_NP = ml_dtypes.bfloat16


def _build_program():
    nc = bacc.Bacc("TRN2", target_bir_lowering=False, debug=False,
                   num_devices=NCORES)
    xt = nc.dram_tensor("xt", [C, BT], BF16, kind="ExternalInput").ap()
    wqkv = nc.dram_tensor("wqkv", [C, 3 * HC], BF16, kind="ExternalInput").ap()
    wproj = nc.dram_tensor("wproj", [C, C], BF16, kind="ExternalInput").ap()
    bq = nc.dram_tensor("bq", [HC, 1], F32, kind="ExternalInput").ap()
    bk = nc.dram_tensor("bk", [HC, 1], F32, kind="ExternalInput").ap()
    bvb = nc.dram_tensor("bvb", [128, HC], BF16, kind="ExternalInput").ap()
    bpb = nc.dram_tensor("bpb", [128, C], BF16, kind="ExternalInput").ap()
    mask2 = nc.dram_tensor("mask2", [TKT, 2, TKT], BF16,
                           kind="ExternalInput").ap()
    outp = nc.dram_tensor("out", [ROWS, C], F32, kind="ExternalOutput").ap()

    with tile.TileContext(nc) as tc:
        with (
            tc.tile_pool(name="consts", bufs=1) as consts,
            tc.tile_pool(name="xpool", bufs=2) as xpool,
            tc.tile_pool(name="ppool", bufs=4) as ppool,
            tc.tile_pool(name="apool", bufs=3) as apool,
            tc.tile_pool(name="npool", bufs=2) as npool,
            tc.tile_pool(name="opool", bufs=2) as opool,
            tc.tile_pool(name="ps_ss", bufs=2, space="PSUM") as ps_ss,
            tc.tile_pool(name="ps_y", bufs=2, space="PSUM") as ps_y,
            tc.tile_pool(name="ps_sum", bufs=1, space="PSUM") as ps_sum,
            tc.tile_pool(name="ps_o", bufs=1, space="PSUM") as ps_o,
            tc.tile_pool(name="dram", bufs=1, space="DRAM") as dram,
        ):
            # ---- stage 0: weights & constants ----
            bq_sb = consts.tile([HC, 1], F32, name="bq_sb")
            nc.sync.dma_start(out=bq_sb, in_=bq)
            bk_sb = consts.tile([HC, 1], F32, name="bk_sb")
            nc.sync.dma_start(out=bk_sb, in_=bk)
            bvb_sb = consts.tile([128, HC], BF16, name="bvb_sb")
            nc.sync.dma_start(out=bvb_sb, in_=bvb)
            # wqkv split per-section, q/k interleaved per kt so the first
            # matmuls can start as early as possible
            wqkv_sb = []
            for kt in range(KT):
                w1 = consts.tile([128, 3 * HC], BF16, name=f"wqkv_sb{kt}")
                wqkv_sb.append(w1)
            for kt in range(KT):
                nc.sync.dma_start(out=wqkv_sb[kt][:, 0:HC],
                                  in_=wqkv[128 * kt:128 * (kt + 1), 0:HC])
                nc.sync.dma_start(out=wqkv_sb[kt][:, HC:2 * HC],
                                  in_=wqkv[128 * kt:128 * (kt + 1), HC:2 * HC])
            for kt in range(KT):
                nc.sync.dma_start(out=wqkv_sb[kt][:, 2 * HC:3 * HC],
                                  in_=wqkv[128 * kt:128 * (kt + 1),
                                           2 * HC:3 * HC])
            # prefetch all xt chunk-pairs: even pairs on the sync queue,
            # odd pairs on the gpsimd queue (idle during stage 1)
            xt_pair = []
            for p in range(4):
                eng = nc.sync if p % 2 == 0 else nc.gpsimd
                xx = []
                for kt in range(KT):
                    x1 = xpool.tile([128, 2 * TQ], BF16, tag=f"xt{kt}",
                                    name=f"xt{kt}_{p}")
                    eng.dma_start(
                        out=x1,
                        in_=xt[128 * kt:128 * (kt + 1),
                               1024 * p:1024 * p + 1024])
                    xx.append(x1)
                xt_pair.append(xx)
            # big weights not needed until late: after the xt prefetches
            wproj_sb = []
            for kt in range(KT):
                w2 = consts.tile([128, C], BF16, name=f"wproj_sb{kt}")
                nc.gpsimd.dma_start(out=w2, in_=wproj[128 * kt:128 * (kt + 1), :])
                wproj_sb.append(w2)
            bpb_sb = consts.tile([128, C], BF16, name="bpb_sb")
            nc.gpsimd.dma_start(out=bpb_sb, in_=bpb)
            mask_sb = consts.tile([TKT, 2, TKT], BF16, name="mask_sb")
            nc.gpsimd.dma_start(out=mask_sb, in_=mask2)
            ones64 = consts.tile([128, D], BF16, name="ones64")
            nc.vector.memset(ones64, 1.0)

            qT_b = [consts.tile([HC, T], BF16, name=f"qT_sb{b}")
                    for b in range(B)]
            kT_b = [consts.tile([HC, T], BF16, name=f"kT_sb{b}")
                    for b in range(B)]
            # v tiles: [128 keys, 128] = [v_h0 (64) | v_h1 (64)]
            v_sb = [consts.tile([128, HC], BF16, name=f"v_sb{tt}")
                    for tt in range(BT // 128)]

            # per-chunk exchange buffers: block s of chunk c = queries
            # [64s, 64s+64) of that chunk, owned by core s
            a2a_in = [dram.tile([NCORES, HC, D], BF16, name=f"a2a_in{c}")
                      for c in range(NCH)]
            a2a_out = [dram.tile([NCORES, HC, D], BF16, name=f"a2a_out{c}")
                       for c in range(NCH)]

            # ---- stage 1: QKV projection (both batches up front) ----
            for b in range(B):
                for cp in range(2):                      # chunk pairs
                    xx = xt_pair[2 * b + cp]
                    for half in range(2):
                        cl = 2 * cp + half               # chunk in batch
                        c = 4 * b + cl                   # global chunk
                        col = slice(TQ * half, TQ * (half + 1))
                        ps = ps_ss.tile([128, 2 * TQ], F32, tag="ss")
                        for kt in range(KT):
                            nc.tensor.matmul(
                                ps[:, 0:TQ],
                                lhsT=wqkv_sb[kt][:, 0:HC],
                                rhs=xx[kt][:, col],
                                start=(kt == 0), stop=(kt == KT - 1))
                        for kt in range(KT):
                            nc.tensor.matmul(
                                ps[:, TQ:2 * TQ],
                                lhsT=wqkv_sb[kt][:, HC:2 * HC],
                                rhs=xx[kt][:, col],
                                start=(kt == 0), stop=(kt == KT - 1))
                        nc.scalar.activation(
                            out=qT_b[b][:, TQ * cl:TQ * (cl + 1)],
                            in_=ps[:, 0:TQ],
                            func=mybir.ActivationFunctionType.Identity,
                            bias=bq_sb, scale=1.0)
                        nc.scalar.activation(
                            out=kT_b[b][:, TQ * cl:TQ * (cl + 1)],
                            in_=ps[:, TQ:2 * TQ],
                            func=mybir.ActivationFunctionType.Identity,
                            bias=bk_sb, scale=SM_SCALE)
                        for s in range(TQ // 128):
                            tt = 4 * c + s
                            pv = ps_sum.tile([128, HC], F32, tag="sum")
                            for kt in range(KT):
                                nc.tensor.matmul(
                                    pv,
                                    lhsT=xx[kt][:, TQ * half + 128 * s:
                                                TQ * half + 128 * (s + 1)],
                                    rhs=wqkv_sb[kt][:, 2 * HC:3 * HC],
                                    start=(kt == 0), stop=(kt == KT - 1))
                            nc.vector.tensor_add(v_sb[tt], pv, bvb_sb)

            # ---- stage 2: attention; two chunks in flight, biggest first;
            # each pair's exchange + output projection follows immediately
            # and hides under the next pair's attention ----
            def emit_S(b, jl, slot):
                """Issue both heads' score matmuls + exp (+ mask) for key
                tile `slot` of chunk jl; returns (pt tile, c0)."""
                i = slot
                q0 = TQ * jl
                k0 = TKT * i
                r = i - (TQ // TKT) * jl
                c0 = TKT * r if r >= 0 else 0
                ss = ps_ss.tile([128, 2 * TQ], F32, tag="ss")
                nc.tensor.matmul(
                    ss[:, c0:TQ],
                    lhsT=kT_b[b][0:D, k0:k0 + TKT],
                    rhs=qT_b[b][0:D, q0 + c0:q0 + TQ],
                    start=True, stop=True)
                nc.tensor.matmul(
                    ss[:, TQ + c0:2 * TQ],
                    lhsT=kT_b[b][D:2 * D, k0:k0 + TKT],
                    rhs=qT_b[b][D:2 * D, q0 + c0:q0 + TQ],
                    start=True, stop=True)
                pt = ppool.tile([128, 2, TQ], BF16, tag="pt")
                ssv = ss.rearrange("p (h q) -> p h q", h=2)
                nc.scalar.activation(
                    out=pt[:, :, c0:TQ], in_=ssv[:, :, c0:TQ],
                    func=mybir.ActivationFunctionType.Exp)
                if r >= 0:
                    nc.vector.tensor_mul(pt[:, :, c0:c0 + TKT],
                                         pt[:, :, c0:c0 + TKT], mask_sb)
                return pt, c0

            def emit_PV(b, jl, i, nkt, y_ps, acc, pt, c0):
                vt = v_sb[(T // 128) * b + i]
                ptf = pt.rearrange("p h q -> p (h q)")
                nc.tensor.matmul(
                    y_ps[0:D, c0:TQ],
                    lhsT=vt[:, 0:D],
                    rhs=ptf[:, c0:TQ],
                    start=(i == 0), stop=(i == nkt - 1),
                    skip_group_check=True)
                nc.tensor.matmul(
                    y_ps[D:2 * D, c0:TQ],
                    lhsT=vt[:, D:2 * D],
                    rhs=ptf[:, TQ + c0:2 * TQ],
                    start=(i == 0), stop=(i == nkt - 1),
                    skip_group_check=True)
                if i == 0:
                    nc.vector.tensor_copy(out=acc, in_=ptf)
                else:
                    accv = acc.rearrange("p (h q) -> p h q", h=2)
                    nc.vector.tensor_add(accv[:, :, c0:TQ],
                                         accv[:, :, c0:TQ],
                                         pt[:, :, c0:TQ])

            def finalize(b, jl, cidx, y_ps, acc):
                sums = ps_sum.tile([128, TQ], F32, tag="sum")
                nc.tensor.matmul(sums[0:D, :], lhsT=ones64,
                                 rhs=acc[:, 0:TQ], start=True, stop=True)
                nc.tensor.matmul(sums[D:2 * D, :], lhsT=ones64,
                                 rhs=acc[:, TQ:2 * TQ], start=True, stop=True)
                recip = npool.tile([128, TQ], F32, tag="recip")
                nc.vector.reciprocal_approx_fast(out=recip, in_=sums)
                yt = npool.tile([128, TQ], BF16, tag="yt")
                nc.vector.tensor_mul(yt, y_ps, recip)
                nc.gpsimd.dma_start(
                    out=a2a_in[cidx].rearrange("s p q -> p s q"),
                    in_=yt.rearrange("p (s q) -> p s q", s=SPC))
                nc.gpsimd.collective_compute(
                    "AllToAll", mybir.AluOpType.bypass,
                    replica_groups=[list(range(NCORES))],
                    ins=[a2a_in[cidx].opt()],
                    outs=[a2a_out[cidx].opt()])

            def stage4_pair(cA, cB):
                """Output projection for two 64-row strips, M=128-packed."""
                yy = opool.tile([128, KT, 2 * D], BF16, tag="ylhs")
                nc.gpsimd.dma_start(
                    out=yy[:, :, 0:D],
                    in_=a2a_out[cA].rearrange("k p q -> p k q"))
                nc.gpsimd.dma_start(
                    out=yy[:, :, D:2 * D],
                    in_=a2a_out[cB].rearrange("k p q -> p k q"))
                for n in range(C // TQ):
                    po = ps_o.tile([128, TQ], F32, tag="po")
                    for kt in range(KT):
                        nc.tensor.matmul(
                            po,
                            lhsT=yy[:, kt, :],
                            rhs=wproj_sb[kt][:, TQ * n:TQ * (n + 1)],
                            start=(kt == 0), stop=(kt == KT - 1))
                    osb = opool.tile([128, TQ], F32, tag="osb")
                    nc.vector.tensor_add(osb, po,
                                         bpb_sb[:, TQ * n:TQ * (n + 1)])
                    for ci, cc in enumerate((cA, cB)):
                        nc.sync.dma_start(
                            out=outp[D * cc:D * (cc + 1), TQ * n:TQ * (n + 1)],
                            in_=osb[D * ci:D * (ci + 1), :])

            for b in range(B):
                for jA, jB in ((3, 0), (2, 1)):
                    cA, cB = 4 * b + jA, 4 * b + jB
                    nktA, nktB = 4 * (jA + 1), 4 * (jB + 1)
                    yA = ps_y.tile([128, TQ], F32, tag="y", name="yA")
                    yB = ps_y.tile([128, TQ], F32, tag="y", name="yB")
                    accA = apool.tile([128, 2 * TQ], BF16, tag="acc",
                                      name="accA")
                    accB = apool.tile([128, 2 * TQ], BF16, tag="acc",
                                      name="accB")
                    prevA = prevB = None
                    for i in range(nktA):
                        curA = emit_S(b, jA, i)
                        if prevA is not None:
                            emit_PV(b, jA, i - 1, nktA, yA, accA, *prevA)
                        prevA = curA
                        if i < nktB:
                            curB = emit_S(b, jB, i)
                            if prevB is not None:
                                emit_PV(b, jB, i - 1, nktB, yB, accB, *prevB)
                            prevB = curB
                        elif i == nktB:
                            # B is done: flush its last PV and kick off its
                            # exchange immediately so the CC stream stays fed
                            emit_PV(b, jB, nktB - 1, nktB, yB, accB, *prevB)
                            finalize(b, jB, cB, yB, accB)
                    emit_PV(b, jA, nktA - 1, nktA, yA, accA, *prevA)
                    finalize(b, jA, cA, yA, accA)
                    stage4_pair(cA, cB)

    nc.compile()
    return nc


_lock = threading.Lock()
_cached_nc = None
last_results = None  # BassKernelResults of the most recent kernel() call


def _get_program():
    global _cached_nc
    with _lock:
        if _cached_nc is None:
            _cached_nc = _build_program()
    return _cached_nc


def _host_inputs(x, W_qkv, b_qkv, W_proj, b_proj):
    bf = lambda a: np.ascontiguousarray(a).astype(BF16_NP)
    x = np.asarray(x, dtype=np.float32)
    W_qkv = np.asarray(W_qkv, dtype=np.float32)
    b_qkv = np.asarray(b_qkv, dtype=np.float32)
    W_proj = np.asarray(W_proj, dtype=np.float32)
    b_proj = np.asarray(b_proj, dtype=np.float32)

    xt = bf(x.reshape(BT, C).T)                     # [C, BT]
    wproj = bf(W_proj)                              # [C, C]
    bpb = bf(np.broadcast_to(b_proj.reshape(1, C), (128, C)))
    k = np.arange(TKT)[:, None, None]
    q = np.arange(TKT)[None, None, :]
    mask2 = np.broadcast_to(k <= q, (TKT, 2, TKT)).astype(BF16_NP)

    in_maps = []
    for i in range(NCORES):
        sel = slice(HC * i, HC * (i + 1))
        wq = W_qkv[:, sel]
        wk = W_qkv[:, C + HC * i:C + HC * (i + 1)]
        wv = W_qkv[:, 2 * C + HC * i:2 * C + HC * (i + 1)]
        bv = b_qkv[2 * C + HC * i:2 * C + HC * (i + 1)]
        in_maps.append({
            "xt": xt,
            "wqkv": bf(np.concatenate([wq, wk, wv], axis=1)),
            "wproj": wproj,
            "bq": np.ascontiguousarray(
                b_qkv[sel].reshape(HC, 1)).astype(np.float32),
            "bk": np.ascontiguousarray(
                (b_qkv[C + HC * i:C + HC * (i + 1)] * SM_SCALE)
                .reshape(HC, 1)).astype(np.float32),
            "bvb": bf(np.broadcast_to(bv.reshape(1, HC), (128, HC))),
            "bpb": bpb,
            "mask2": np.ascontiguousarray(mask2),
        })
    return in_maps


def kernel(x, W_qkv, b_qkv, W_proj, b_proj):
    global last_results
    nc = _get_program()
    in_maps = _host_inputs(x, W_qkv, b_qkv, W_proj, b_proj)
    trace = bool(int(os.environ.get("KERNEL_TRACE", "0")))
    res = bass_utils.run_bass_kernel_spmd(
        nc, in_maps, core_ids=list(range(NCORES)), trace=trace)
    last_results = res
    # core s's output rows are strip s (64 rows) of every 512-row chunk
    arr = np.stack([res.results[s]["out"].reshape(BT // TQ, D, C)
                    for s in range(NCORES)], axis=1)   # [chunk, core, 64, C]
    return np.ascontiguousarray(arr.reshape(B, T, C))
